# revision 1
# baseline (speedup 1.0000x reference)
"""GATv2 (2-layer) + global-mean-pool + MLP head on 8 Trainium2 NeuronCores.

Self-contained: host preprocessing (numpy) + Bass/Tile program + SPMD run.

Strategy (dst-sharded graph parallel):
  - Nodes/edges sharded across 8 cores by destination-node windows of 128.
  - Layer math is folded so each edge needs ONE gathered fp16 table row:
      table row n = [att*xl[n] (feature-permuted, +att first) | att.xl[n] | 1 | pad]
    using leaky_relu(z,0.2) = 0.6 z + 0.4 |z|:
      e = 0.6(pl_s + pr_d) + 0.4(sum_{att>0}|u| - sum_{att<=0}|u|),  u = att*m
    The aggregation runs on att-scaled rows and un-scales by 1/att afterwards.
  - xr[dst] per edge is expanded on-chip with one-hot matmuls (no gather);
    the transposed one-hot comes from a partition-broadcast DMA + is_equal.
  - Self-loop edges form chunk 0 of every window and are served from a
    locally-computed xl window tile (no gather DMA).
  - exp() without segment-max (validated: e in [-2.6, 3.6]); padding edges get
    a -60 exponent bias; denominators are clamped before reciprocal.
  - Layer 1 node transforms are computed fully replicated (x is replicated) so
    layer 1 needs NO collective. Between layers one AllGather of hx2T (fp16);
    final graph sums via AllReduce of a [128f x 128g] tile; FC head replicated.
"""
import numpy as np

P = 128          # partitions / window size / chunk size
TW = 132         # table row width: 128 feats + pl + 1.0 + 2 pad

FULL_CFG = dict(N=100000, DIN=64, H=128, E0=600000, G=128, DOUT=16, NCORES=8)


# ---------------------------------------------------------------------------
# host preprocessing
# ---------------------------------------------------------------------------

def _sign_perm(att):
    pos = np.where(att > 0)[0]
    neg = np.where(att <= 0)[0]
    return np.concatenate([pos, neg]), len(pos)


def _aug_weights(W, b, att, perm, attp):
    H = W.shape[0]
    Wa = np.zeros((H, TW), np.float32)
    ba = np.zeros((TW,), np.float32)
    Wa[:, :H] = W[:, perm] * attp[None, :]
    ba[:H] = b[perm] * attp
    Wa[:, H] = W @ att
    ba[H] = att @ b
    ba[H + 1] = 1.0
    return Wa, ba


def host_prep(inputs, cfg):
    N, DIN, H, E0, G, NCORES = (cfg["N"], cfg["DIN"], cfg["H"], cfg["E0"],
                                cfg["G"], cfg["NCORES"])
    x = np.asarray(inputs["x"], np.float32)
    ei = np.asarray(inputs["edge_index"]).astype(np.int64)
    batch = np.asarray(inputs["batch"]).astype(np.int64)
    get = lambda k: np.asarray(inputs[k], np.float32)
    f16 = np.float16

    NWT = (N + P - 1) // P
    NWC = (NWT + NCORES - 1) // NCORES
    NLOC = NWC * P
    NPAD = NLOC * NCORES

    att1, att2 = get("att1"), get("att2")
    perm1, Pp1 = _sign_perm(att1)
    perm2, Pp2 = _sign_perm(att2)
    attp1 = att1[perm1].astype(np.float32)
    attp2 = att2[perm2].astype(np.float32)

    Wl1a, bl1a = _aug_weights(get("Wl1"), get("bl1"), att1, perm1, attp1)
    Wr1a, br1a = _aug_weights(get("Wr1"), get("br1"), att1, perm1, attp1)
    br1a[H + 1] = 0.0
    Wl2a, bl2a = _aug_weights(get("Wl2")[perm1, :], get("bl2"), att2, perm2, attp2)
    Wr2a, br2a = _aug_weights(get("Wr2")[perm1, :], get("br2"), att2, perm2, attp2)
    br2a[H + 1] = 0.0

    weights = {
        "wnfc": get("W_nfc").astype(f16),
        "bnfc": get("b_nfc").reshape(H, 1),
        "wl1a": Wl1a.astype(f16), "wr1a": Wr1a.astype(f16),
        "bl1B": np.tile(bl1a, (P, 1)), "br1B": np.tile(br1a, (P, 1)),
        "wl2a": Wl2a.astype(f16), "wr2a": Wr2a.astype(f16),
        "bl2B": np.tile(bl2a, (P, 1)), "br2B": np.tile(br2a, (P, 1)),
        "arec1B": np.tile((1.0 / attp1), (P, 1)),
        "arec2B": np.tile((1.0 / attp2), (P, 1)),
        "bias1B": np.tile(get("bias1")[perm1], (P, 1)),
        "bias2B": np.tile(get("bias2")[perm2], (P, 1)),
        "wfc1": get("W_fc1")[perm2, :].astype(f16),
        "bfc1": get("b_fc1").reshape(1, -1).astype(f16),
        "wfc2": get("W_fc2").astype(f16),
        "bfc2": get("b_fc2").reshape(1, -1).astype(f16),
    }
    assert np.abs(attp1).min() > 1e-12 and np.abs(attp2).min() > 1e-12

    xT = np.zeros((DIN, NPAD), f16)
    xT[:, :N] = x.T.astype(f16)
    xTloc = [np.ascontiguousarray(xT[:, c * NLOC:(c + 1) * NLOC])
             for c in range(NCORES)]

    # --- edges (self-loops are NOT appended; they become chunk 0/window) ---
    src0, dst0 = ei[0], ei[1]
    per_core = []
    for c in range(NCORES):
        lo, hi = c * NLOC, min((c + 1) * NLOC, N)
        sel = (dst0 >= lo) & (dst0 < hi)
        s, d = src0[sel], dst0[sel]
        o = np.argsort(d, kind="stable")
        per_core.append((s[o], d[o] - lo))

    cnt = np.zeros((NCORES, NWC), np.int64)
    for c in range(NCORES):
        _, dl = per_core[c]
        cnt[c] = np.bincount(dl // P, minlength=NWC)
    # chunk 0 = self chunk; then real-edge chunks
    cw = 1 + np.ceil(cnt.max(axis=0) / P).astype(np.int64)      # [NWC]
    k0 = np.concatenate([[0], np.cumsum(cw)])
    K = int(k0[-1])
    CWMAX = int(cw.max())

    src_idx = np.zeros((NCORES, P, K), np.int32)
    dst_rel = np.zeros((NCORES, P, K), f16)
    ebp = np.full((NCORES, P, K), -150.0, np.float32)
    for c in range(NCORES):
        lo, hi = c * NLOC, min((c + 1) * NLOC, N)
        s, dl = per_core[c]
        w = dl // P
        starts = np.searchsorted(w, np.arange(NWC), side="left")
        ends = np.searchsorted(w, np.arange(NWC), side="right")
        for wi in range(NWC):
            # self chunk: slot p -> node p of window (real nodes only)
            nreal = max(0, min(hi - (lo + wi * P), P))
            pr = np.arange(P)
            dst_rel[c, pr, k0[wi]] = pr.astype(f16)
            ebp[c, pr[:nreal], k0[wi]] = 0.0
            # real edges from chunk k0[wi]+1
            a, b = int(starts[wi]), int(ends[wi])
            n = b - a
            j = np.arange(n)
            ch = k0[wi] + 1 + j // P
            pr = j % P
            src_idx[c, pr, ch] = s[a:b]
            dst_rel[c, pr, ch] = (dl[a:b] % P).astype(f16)
            ebp[c, pr, ch] = 0.0
            # pad edges spread over rel-slots
            slots = (int(cw[wi]) - 1) * P
            j = np.arange(n, slots)
            ch = k0[wi] + 1 + j // P
            pr = j % P
            src_idx[c, pr, ch] = 0
            dst_rel[c, pr, ch] = (j % P).astype(f16)

    # edge-major flat dst_rel for the broadcast-DMA transposed one-hot
    dst_flat = np.zeros((NCORES, 1, K * P), f16)
    for c in range(NCORES):
        dst_flat[c, 0, :] = dst_rel[c].T.reshape(-1)

    gmask = np.zeros((NCORES, P, NWC * G), f16)
    for c in range(NCORES):
        lo, hi = c * NLOC, min((c + 1) * NLOC, N)
        for wi in range(NWC):
            nlo = lo + wi * P
            nn = max(0, min(hi - nlo, P))
            if nn <= 0:
                continue
            gmask[c, np.arange(nn), wi * G + batch[nlo:nlo + nn]] = 1.0
    counts = np.bincount(batch, minlength=G).astype(np.float32)
    countsRecipB = np.tile(1.0 / np.maximum(counts, 1.0), (P, 1)).astype(np.float32)

    meta = dict(cfg=cfg, NWC=NWC, NLOC=NLOC, NPAD=NPAD, K=K, CWMAX=CWMAX,
                cw=cw.tolist(), k0=k0.tolist(), Pp1=Pp1, Pp2=Pp2)
    data = dict(weights=weights, xT=xT, xTloc=xTloc, src_idx=src_idx,
                dst_rel=dst_rel, dst_flat=dst_flat, ebp=ebp, gmask=gmask,
                countsRecipB=countsRecipB)
    return meta, data


# ---------------------------------------------------------------------------
# device program
# ---------------------------------------------------------------------------

def build_program(meta):
    import concourse.bass as bass
    import concourse.bacc as bacc
    import concourse.tile as tile
    import concourse.mybir as mybir
    from concourse.masks import make_identity

    cfg = meta["cfg"]
    N, DIN, H, G, DOUT, NCORES = (cfg["N"], cfg["DIN"], cfg["H"], cfg["G"],
                                  cfg["DOUT"], cfg["NCORES"])
    NWC, NLOC, NPAD, K, CWMAX = (meta["NWC"], meta["NLOC"], meta["NPAD"],
                                 meta["K"], meta["CWMAX"])
    cw, k0 = meta["cw"], meta["k0"]
    NT = NPAD // P
    f32 = mybir.dt.float32
    f16 = mybir.dt.float16
    AF = mybir.ActivationFunctionType
    OP = mybir.AluOpType

    nc = bacc.Bacc("TRN2", target_bir_lowering=False, debug=False,
                   num_devices=NCORES)

    d_xT = nc.dram_tensor("xT", [DIN, NPAD], f16, kind="ExternalInput")
    d_xTloc = nc.dram_tensor("xTloc", [DIN, NLOC], f16, kind="ExternalInput")
    d_src = nc.dram_tensor("src_idx", [P, K], mybir.dt.int32, kind="ExternalInput")
    d_dst = nc.dram_tensor("dst_rel", [P, K], f16, kind="ExternalInput")
    d_dstf = nc.dram_tensor("dst_flat", [1, K * P], f16, kind="ExternalInput")
    d_ebp = nc.dram_tensor("ebp", [P, K], f32, kind="ExternalInput")
    d_gmask = nc.dram_tensor("gmask", [P, NWC * G], f16, kind="ExternalInput")
    d_crecip = nc.dram_tensor("countsRecipB", [P, G], f32, kind="ExternalInput")
    wnames = {
        "wnfc": ([DIN, H], f16), "bnfc": ([H, 1], f32),
        "wl1a": ([H, TW], f16), "wr1a": ([H, TW], f16),
        "bl1B": ([P, TW], f32), "br1B": ([P, TW], f32),
        "wl2a": ([H, TW], f16), "wr2a": ([H, TW], f16),
        "bl2B": ([P, TW], f32), "br2B": ([P, TW], f32),
        "arec1B": ([P, H], f32), "arec2B": ([P, H], f32),
        "bias1B": ([P, H], f32), "bias2B": ([P, H], f32),
        "wfc1": ([H, 32], f16), "bfc1": ([1, 32], f16),
        "wfc2": ([32, DOUT], f16), "bfc2": ([1, DOUT], f16),
    }
    d_w = {k: nc.dram_tensor(k, shp, dt, kind="ExternalInput")
           for k, (shp, dt) in wnames.items()}
    d_out = nc.dram_tensor("out", [G, DOUT], f32, kind="ExternalOutput")

    d_tab1 = nc.dram_tensor("tab1", [NPAD, TW], f16, kind="Internal")
    d_tab2 = nc.dram_tensor("tab2", [NPAD, TW], f16, kind="Internal")
    d_h2loc = nc.dram_tensor("h2loc", [P, NLOC], f16, kind="Internal")
    d_h2full = nc.dram_tensor("h2full", [NCORES, P, NLOC], f16,
                              kind="Internal", addr_space="Shared")
    d_gsin = nc.dram_tensor("gsin", [P, G], f32, kind="Internal")
    d_gsout = nc.dram_tensor("gsout", [P, G], f32, kind="Internal",
                             addr_space="Shared")

    def bcast_last(ap2d, c, j):
        return bass.AP(ap2d.tensor, ap2d.offset,
                       [list(ap2d.ap[0]), list(ap2d.ap[1]), [0, j]])

    def bcast_mid(ap2d, c):
        return bass.AP(ap2d.tensor, ap2d.offset,
                       [list(ap2d.ap[0]), [0, c], list(ap2d.ap[1])])

    def bcast_row(ap2d, e):
        # [P(=any), 1] -> [P, e] broadcast along free
        return bass.AP(ap2d.tensor, ap2d.offset,
                       [list(ap2d.ap[0]), [0, e]])

    def bcast_part(ap_row, n):
        # [1, E] dram row -> [n, E] partition broadcast
        return bass.AP(ap_row.tensor, ap_row.offset,
                       [[0, n], list(ap_row.ap[1])])

    with tile.TileContext(nc) as tc:
        with tc.tile_pool(name="const", bufs=1) as cpool:
            identf = cpool.tile([P, P], f32)
            make_identity(nc, identf[:, :])
            ident = cpool.tile([P, P], f16)
            nc.vector.tensor_copy(ident[:, :], identf[:, :])
            iotaI = cpool.tile([P, P], mybir.dt.int32)
            nc.gpsimd.iota(iotaI[:, :], pattern=[[1, P]], base=0,
                           channel_multiplier=0)
            iotaF = cpool.tile([P, P], f16)
            nc.vector.tensor_copy(iotaF[:, :], iotaI[:, :])
            iotaPI = cpool.tile([P, 1], mybir.dt.int32)
            nc.gpsimd.iota(iotaPI[:, :], pattern=[[1, 1]], base=0,
                           channel_multiplier=1)
            iotaP = cpool.tile([P, 1], f16)
            nc.vector.tensor_copy(iotaP[:, :], iotaPI[:, :])
            ones1 = cpool.tile([1, P], f16)
            nc.vector.memset(ones1[:, :], 1.0)

            w_sb = {}
            for k, (shp, dt) in wnames.items():
                w_sb[k] = cpool.tile(shp, dt, name=f"w_{k}", tag=f"w_{k}")
                nc.sync.dma_start(out=w_sb[k][:, :], in_=d_w[k][:, :])
            src_sb = cpool.tile([P, K], mybir.dt.int32)
            nc.sync.dma_start(out=src_sb[:, :], in_=d_src[:, :])
            dst_sb = cpool.tile([P, K], f16)
            nc.sync.dma_start(out=dst_sb[:, :], in_=d_dst[:, :])
            ebp_sb = cpool.tile([P, K], f32)
            nc.sync.dma_start(out=ebp_sb[:, :], in_=d_ebp[:, :])
            crecip_sb = cpool.tile([P, G], f32)
            nc.sync.dma_start(out=crecip_sb[:, :], in_=d_crecip[:, :])

            with tc.tile_pool(name="big", bufs=1) as bigp:
                xr_sb = bigp.tile([P, NWC * TW], f16, tag="xr")
                xl_sb = bigp.tile([P, NWC * TW], f16, tag="xl")
                h2T_sb = bigp.tile([P, NLOC], f16, tag="h2T")

                # ============ table build (batched 4-tile DMA writes) =====
                def build_table(layer, d_tab, lhsT_src):
                    wla = w_sb["wl1a" if layer == 1 else "wl2a"]
                    blB = w_sb["bl1B" if layer == 1 else "bl2B"]
                    blB4 = bass.AP(blB[:, :].tensor, blB[:, :].offset,
                                   [list(blB[:, :].ap[0]), [0, 4],
                                    list(blB[:, :].ap[1])])
                    with (
                        tc.tile_pool(name=f"tA{layer}", bufs=3) as sp,
                        tc.tile_pool(name=f"tAps{layer}", bufs=2,
                                     space="PSUM") as pp,
                    ):
                        for t0 in range(0, NT, 4):
                            nt = min(4, NT - t0)
                            tabg = sp.tile([P, 4 * TW], f16, tag="tabg")
                            psg = pp.tile([P, 4 * 512], f32, tag="tab", bufs=1)
                            psg3 = psg[:, :].rearrange("p (t f) -> p t f", f=512)
                            for i in range(nt):
                                lhsT = lhsT_src(t0 + i, sp, pp)
                                nc.tensor.matmul(out=psg3[:, i, 0:TW], lhsT=lhsT,
                                                 rhs=wla[:, :],
                                                 start=True, stop=True)
                            nc.vector.scalar_tensor_tensor(
                                out=tabg[:, :nt * TW].rearrange(
                                    "p (t f) -> p t f", f=TW),
                                in0=psg3[:, 0:nt, 0:TW], scalar=1.0,
                                in1=blB4[:, 0:nt, :], op0=OP.mult, op1=OP.add)
                            dv = d_tab[t0 * P:(t0 + nt) * P, :]
                            dv3 = dv.rearrange("(t p) f -> p t f", p=P)
                            nc.sync.dma_start(
                                out=dv3,
                                in_=tabg[:, :nt * TW].rearrange(
                                    "p (t f) -> p t f", f=TW))

                hxg_cache = {}

                def l1_lhsT(t, sp, pp):
                    gidx = t // 4
                    if gidx not in hxg_cache:
                        xg = sp.tile([DIN, 512], f16, tag="xg")
                        nc.sync.dma_start(out=xg[:, :],
                                          in_=d_xT[:, gidx * 512:(gidx + 1) * 512])
                        psn = pp.tile([P, 512], f32, tag="nfc")
                        nc.tensor.matmul(out=psn[:, :], lhsT=w_sb["wnfc"][:, :],
                                         rhs=xg[:, :], start=True, stop=True)
                        hxg = sp.tile([P, 512], f16, tag="hxg")
                        nc.scalar.activation(out=hxg[:, :], in_=psn[:, :],
                                             func=AF.Lrelu,
                                             bias=w_sb["bnfc"][:, :],
                                             scale=1.0, alpha=0.01)
                        hxg_cache.clear()
                        hxg_cache[gidx] = hxg
                    s = (t % 4) * P
                    return hxg_cache[gidx][:, s:s + P]

                build_table(1, d_tab1, l1_lhsT)
                hxg_cache.clear()

                # local window products: xr (Wr) and xl (Wl, for self chunks)
                def build_loc(layer, hx_lhsT):
                    wra = w_sb["wr1a" if layer == 1 else "wr2a"]
                    brB = w_sb["br1B" if layer == 1 else "br2B"]
                    wla = w_sb["wl1a" if layer == 1 else "wl2a"]
                    blB = w_sb["bl1B" if layer == 1 else "bl2B"]
                    with (
                        tc.tile_pool(name=f"xr{layer}", bufs=3) as sp,
                        tc.tile_pool(name=f"xrps{layer}", bufs=2,
                                     space="PSUM") as pp,
                    ):
                        for w in range(NWC):
                            lhsT = hx_lhsT(w, sp, pp)
                            ps = pp.tile([P, TW], f32, tag="xr")
                            nc.tensor.matmul(out=ps[:, :], lhsT=lhsT,
                                             rhs=wra[:, :], start=True, stop=True)
                            nc.vector.scalar_tensor_tensor(
                                out=xr_sb[:, w * TW:(w + 1) * TW], in0=ps[:, :],
                                scalar=1.0, in1=brB[:, :],
                                op0=OP.mult, op1=OP.add)
                            ps2 = pp.tile([P, TW], f32, tag="xl")
                            nc.tensor.matmul(out=ps2[:, :], lhsT=lhsT,
                                             rhs=wla[:, :], start=True, stop=True)
                            nc.vector.scalar_tensor_tensor(
                                out=xl_sb[:, w * TW:(w + 1) * TW], in0=ps2[:, :],
                                scalar=1.0, in1=blB[:, :],
                                op0=OP.mult, op1=OP.add)

                lhx_cache = {}

                def l1_loc_lhsT(w, sp, pp):
                    gidx = w // 4
                    if gidx not in lhx_cache:
                        g0 = gidx * 512
                        gl = min(512, NLOC - g0)
                        xg = sp.tile([DIN, 512], f16, tag="xgl")
                        nc.sync.dma_start(out=xg[:, :gl],
                                          in_=d_xTloc[:, g0:g0 + gl])
                        psn = pp.tile([P, 512], f32, tag="nfcl")
                        nc.tensor.matmul(out=psn[:, :gl],
                                         lhsT=w_sb["wnfc"][:, :],
                                         rhs=xg[:, :gl], start=True, stop=True)
                        hxg = sp.tile([P, 512], f16, tag="hxgl")
                        nc.scalar.activation(out=hxg[:, :gl], in_=psn[:, :gl],
                                             func=AF.Lrelu,
                                             bias=w_sb["bnfc"][:, :],
                                             scale=1.0, alpha=0.01)
                        lhx_cache.clear()
                        lhx_cache[gidx] = hxg
                    s = (w % 4) * P
                    return lhx_cache[gidx][:, s:s + P]

                build_loc(1, l1_loc_lhsT)
                lhx_cache.clear()

                # ============ edge phase ============
                def edge_phase(layer, d_tab, Pp, h_out_cb):
                    arecB = w_sb["arec1B" if layer == 1 else "arec2B"]
                    biasB = w_sb["bias1B" if layer == 1 else "bias2B"]
                    with (
                        tc.tile_pool(name=f"eg{layer}", bufs=3 * CWMAX) as gp,
                        tc.tile_pool(name=f"ew{layer}", bufs=3) as sp,
                        tc.tile_pool(name=f"es{layer}", bufs=4) as ssp,
                        tc.tile_pool(name=f"eps{layer}", bufs=2,
                                     space="PSUM") as ppm,
                        tc.tile_pool(name=f"epo{layer}", bufs=2,
                                     space="PSUM") as ppo,
                    ):
                        for w in range(NWC):
                            c = cw[w]
                            ks = k0[w]
                            # chunk 0 rhs = local xl window; others gathered
                            rhs = [xl_sb[:, w * TW:(w + 1) * TW]]
                            for j in range(1, c):
                                gt = gp.tile([P, TW], f16, tag="g")
                                nc.gpsimd.indirect_dma_start(
                                    out=gt[:, :], out_offset=None,
                                    in_=d_tab[:, :],
                                    in_offset=bass.IndirectOffsetOnAxis(
                                        ap=src_sb[:, ks + j:ks + j + 1], axis=0))
                                rhs.append(gt[:, :])
                            # edge-partitioned one-hot [P, c, 128]
                            oh = sp.tile([P, CWMAX * P], f16, tag="oh")
                            oh3 = oh[:, :c * P].rearrange("p (c j) -> p c j", j=P)
                            nc.vector.tensor_tensor(
                                out=oh3,
                                in0=bcast_last(dst_sb[:, ks:ks + c], c, P),
                                in1=bcast_mid(iotaF[:, :], c),
                                op=OP.is_equal)
                            # transposed one-hot via partition-broadcast DMA
                            dstb = sp.tile([P, CWMAX * P], f16, tag="dstb")
                            nc.sync.dma_start(
                                out=dstb[:, :c * P],
                                in_=bcast_part(d_dstf[:, ks * P:(ks + c) * P], P))
                            ohT = sp.tile([P, CWMAX * P], f16, tag="ohT")
                            nc.vector.tensor_tensor(
                                out=ohT[:, :c * P],
                                in0=bcast_row(iotaP[:, :], c * P),
                                in1=dstb[:, :c * P], op=OP.is_equal)
                            # m' per chunk in bank-aligned psum half-windows
                            av = ssp.tile([P, CWMAX], f16, tag="av")
                            for h0 in range(0, c, 4):
                                hc = min(4, c - h0)
                                psm = ppm.tile([P, 4 * 512], f32, tag="m",
                                               bufs=1)
                                psm3 = psm[:, :].rearrange(
                                    "p (t f) -> p t f", f=512)
                                for jj in range(hc):
                                    j = h0 + jj
                                    nc.tensor.matmul(
                                        out=psm3[:, jj, 0:TW],
                                        lhsT=ohT[:, j * P:(j + 1) * P],
                                        rhs=xr_sb[:, w * TW:(w + 1) * TW],
                                        start=True, stop=False)
                                    nc.tensor.matmul(
                                        out=psm3[:, jj, 0:TW], lhsT=ident[:, :],
                                        rhs=rhs[j], start=False, stop=True)
                                rp = ssp.tile([P, 4], f32, tag="rp")
                                nc.vector.tensor_reduce(
                                    out=rp[:, :hc], in_=psm3[:, 0:hc, 0:Pp],
                                    axis=mybir.AxisListType.X, op=OP.add,
                                    apply_absolute_value=True)
                                e0 = ssp.tile([P, 4], f32, tag="e0")
                                if Pp < H:
                                    rn = ssp.tile([P, 4], f32, tag="rn")
                                    nc.vector.tensor_reduce(
                                        out=rn[:, :hc], in_=psm3[:, 0:hc, Pp:H],
                                        axis=mybir.AxisListType.X, op=OP.add,
                                        apply_absolute_value=True)
                                    nc.vector.tensor_tensor(
                                        out=e0[:, :hc], in0=rp[:, :hc],
                                        in1=rn[:, :hc], op=OP.subtract)
                                else:
                                    nc.vector.tensor_copy(e0[:, :hc], rp[:, :hc])
                                e1 = ssp.tile([P, 4], f32, tag="e1")
                                nc.vector.scalar_tensor_tensor(
                                    out=e1[:, :hc], in0=psm3[:, 0:hc, H],
                                    scalar=1.5, in1=e0[:, :hc],
                                    op0=OP.mult, op1=OP.add)
                                e2 = ssp.tile([P, 4], f32, tag="e2")
                                nc.vector.tensor_tensor(
                                    out=e2[:, :hc], in0=e1[:, :hc],
                                    in1=ebp_sb[:, ks + h0:ks + h0 + hc],
                                    op=OP.add)
                                nc.scalar.activation(
                                    out=av[:, h0:h0 + hc], in_=e2[:, :hc],
                                    func=AF.Exp, scale=0.4)
                            oha = sp.tile([P, CWMAX * P], f16, tag="oha")
                            nc.vector.tensor_tensor(
                                out=oha[:, :c * P].rearrange(
                                    "p (c j) -> p c j", j=P),
                                in0=oh3, in1=bcast_last(av[:, :c], c, P),
                                op=OP.mult)
                            pso = ppo.tile([P, H + 2], f32, tag="out")
                            for j in range(c):
                                nc.tensor.matmul(
                                    out=pso[:, :], lhsT=oha[:, j * P:(j + 1) * P],
                                    rhs=rhs[j][:, 0:H + 2],
                                    start=(j == 0), stop=(j == c - 1))
                            dcl = ssp.tile([P, 1], f32, tag="dcl")
                            nc.vector.tensor_scalar_max(dcl[:, :],
                                                        pso[:, H + 1:H + 2],
                                                        1e-20)
                            rd = ssp.tile([P, 1], f32, tag="rd")
                            nc.vector.reciprocal(rd[:, :], dcl[:, :])
                            h1 = ssp.tile([P, H], f32, tag="h1")
                            nc.vector.scalar_tensor_tensor(
                                out=h1[:, :], in0=pso[:, 0:H], scalar=rd[:, :],
                                in1=arecB[:, :], op0=OP.mult, op1=OP.mult)
                            h2 = ssp.tile([P, H], f32, tag="h2")
                            nc.vector.tensor_tensor(
                                out=h2[:, :], in0=h1[:, :], in1=biasB[:, :],
                                op=OP.add)
                            hw_ = ssp.tile([P, H], f16, tag="hw")
                            nc.scalar.activation(out=hw_[:, :], in_=h2[:, :],
                                                 func=AF.Lrelu, alpha=0.01)
                            h_out_cb(w, hw_, ssp, ppm)

                def l1_out(w, hw_, ssp, ppt):
                    psT = ppt.tile([P, P], f16, tag="tr")
                    nc.tensor.transpose(out=psT[:, :], in_=hw_[:, :],
                                        identity=ident[:, :])
                    nc.vector.tensor_copy(h2T_sb[:, w * P:(w + 1) * P],
                                          psT[:, :])

                edge_phase(1, d_tab1, meta["Pp1"], l1_out)

                # ---- exchange hx2T ----
                nc.sync.dma_start(out=d_h2loc[:, :], in_=h2T_sb[:, :])
                nc.gpsimd.collective_compute(
                    "AllGather", OP.bypass,
                    replica_groups=[list(range(NCORES))],
                    ins=[d_h2loc[:, :]], outs=[d_h2full[:, :, :]])

                # ---- layer-2 table ----
                lt_cache = {}

                def l2_lhsT(t, sp, pp):
                    pc, off = t // NWC, t % NWC
                    g4 = off // 4
                    key = (pc, g4)
                    if key not in lt_cache:
                        o0 = g4 * 4 * P
                        ol = min(4 * P, NLOC - o0)
                        lt = sp.tile([P, 4 * P], f16, tag="lhs")
                        nc.sync.dma_start(out=lt[:, :ol],
                                          in_=d_h2full[pc, :, o0:o0 + ol])
                        lt_cache.clear()
                        lt_cache[key] = lt
                    s = (off % 4) * P
                    return lt_cache[key][:, s:s + P]

                build_table(2, d_tab2, l2_lhsT)
                lt_cache.clear()

                def l2_loc_lhsT(w, sp, pp):
                    return h2T_sb[:, w * P:(w + 1) * P]

                build_loc(2, l2_loc_lhsT)

                # ---- layer 2 edge phase + pooling accumulate ----
                with (
                    tc.tile_pool(name="gm", bufs=3) as gmp,
                    tc.tile_pool(name="gps", bufs=1, space="PSUM") as gpsp,
                ):
                    ps_gs = gpsp.tile([P, G], f32, tag="gs")

                    def l2_out(w, hw_, ssp, ppt):
                        gm = gmp.tile([P, G], f16, tag="gm")
                        nc.sync.dma_start(out=gm[:, :],
                                          in_=d_gmask[:, w * G:(w + 1) * G])
                        nc.tensor.matmul(out=ps_gs[:, :], lhsT=hw_[:, :],
                                         rhs=gm[:, :], start=(w == 0),
                                         stop=(w == NWC - 1))

                    edge_phase(2, d_tab2, meta["Pp2"], l2_out)

                    with (
                        tc.tile_pool(name="fc", bufs=1) as fp,
                        tc.tile_pool(name="fcps", bufs=1, space="PSUM") as fpp,
                    ):
                        gsum = fp.tile([P, G], f32)
                        nc.vector.tensor_copy(gsum[:, :], ps_gs[:, :])
                        nc.sync.dma_start(out=d_gsin[:, :], in_=gsum[:, :])
                        nc.gpsimd.collective_compute(
                            "AllReduce", OP.add,
                            replica_groups=[list(range(NCORES))],
                            ins=[d_gsin[:, :]], outs=[d_gsout[:, :]])
                        gsum2 = fp.tile([P, G], f32)
                        nc.sync.dma_start(out=gsum2[:, :], in_=d_gsout[:, :])
                        meanT = fp.tile([P, G], f16)
                        nc.vector.tensor_tensor(out=meanT[:, :], in0=gsum2[:, :],
                                                in1=crecip_sb[:, :], op=OP.mult)
                        psf = fpp.tile([P, 32], f32, tag="f1")
                        nc.tensor.matmul(out=psf[:G, :], lhsT=meanT[:, :G],
                                         rhs=w_sb["wfc1"][:, :],
                                         start=True, stop=False)
                        nc.tensor.matmul(out=psf[:G, :], lhsT=ones1[:, :G],
                                         rhs=w_sb["bfc1"][:, :],
                                         start=False, stop=True)
                        hf1 = fp.tile([P, 32], f16)
                        nc.scalar.activation(out=hf1[:G, :], in_=psf[:G, :],
                                             func=AF.Lrelu, alpha=0.01)
                        psT = fpp.tile([P, P], f16, tag="ft")
                        nc.tensor.transpose(out=psT[:32, :G], in_=hf1[:G, :32],
                                            identity=ident[:G, :G])
                        hf1T = fp.tile([32, P], f16)
                        nc.scalar.activation(out=hf1T[:, :G], in_=psT[:32, :G],
                                             func=AF.Copy)
                        pso = fpp.tile([P, DOUT], f32, tag="f2")
                        nc.tensor.matmul(out=pso[:G, :], lhsT=hf1T[:, :G],
                                         rhs=w_sb["wfc2"][:, :],
                                         start=True, stop=False)
                        nc.tensor.matmul(out=pso[:G, :], lhsT=ones1[:, :G],
                                         rhs=w_sb["bfc2"][:, :],
                                         start=False, stop=True)
                        fout = fp.tile([P, DOUT], f32)
                        nc.vector.tensor_copy(fout[:G, :], pso[:G, :])
                        nc.sync.dma_start(out=d_out[:, :], in_=fout[:G, :])

    nc.compile()
    return nc


# ---------------------------------------------------------------------------
# runner
# ---------------------------------------------------------------------------

def _in_maps(meta, data):
    cfg = meta["cfg"]
    maps = []
    for c in range(cfg["NCORES"]):
        m = {
            "xT": data["xT"],
            "xTloc": data["xTloc"][c],
            "src_idx": data["src_idx"][c],
            "dst_rel": data["dst_rel"][c],
            "dst_flat": data["dst_flat"][c],
            "ebp": data["ebp"][c],
            "gmask": data["gmask"][c],
            "countsRecipB": data["countsRecipB"],
        }
        for k, v in data["weights"].items():
            m[k] = np.ascontiguousarray(v)
        maps.append(m)
    return maps


def run_on_device(inputs, cfg, trace=False):
    from concourse.bass_utils import run_bass_kernel_spmd
    meta, data = host_prep(inputs, cfg)
    nc = build_program(meta)
    res = run_bass_kernel_spmd(nc, _in_maps(meta, data),
                               core_ids=list(range(cfg["NCORES"])), trace=trace)
    return res


def kernel(**inputs):
    res = run_on_device(inputs, FULL_CFG, trace=False)
    return np.asarray(res.results[0]["out"], np.float32)

